# revision 19
# baseline (speedup 1.0000x reference)
"""Trainium2 Bass kernel for sliding-window causal self-attention.

Reference computation (B=1, T=4096, H=8 heads, head_dim=128, DIM=1024):
  qkv = x @ w_qkv.T; q,k = rms_norm -> rope; v = lam0*rms_norm(v) + lam1*ve
  scores = (q k^T) * 0.12 with sliding-window causal mask (0 <= i-j < 512)
  y = softmax(scores) @ v;  out = y @ o_w.T

Sharding over 8 cores: 2 sequence halves (S) x 4 head-pair groups (G).
Core c = 4*s + g handles t in [2048s, 2048(s+1)) for heads {2g, 2g+1}.
Each core reads its x rows plus a 512-row halo of preceding rows (for k/v),
computes its partial output projection over its 2 heads, and the host sums
the 4 partials per half and concatenates the halves. No on-chip collectives.

Attention uses a transposeless [kj, qi] scores layout: q-tiles are processed
in pairs (256 queries, 768-key window, 6 key chunks) so every matmul has a
moving free dim >= 256, which is required for full-rate float32r matmuls.
Softmax runs without max-subtraction (scores are bounded by 0.12*128), the
kj-sum is done on the PE with a ones vector, and the reciprocal is broadcast
across partitions with a rank-1 matmul. The output projection is interleaved
into the attention loop so its DMA overlaps compute. Elementwise work is
spread across DVE / ScalarE / GpSimd to keep all engines busy.
"""

import sys

sys.path.insert(0, "/opt/trn_rl_repo")

import numpy as np

import concourse.bass as bass
import concourse.mybir as mybir
import concourse.tile as tile
from concourse import bacc
from concourse.bass_utils import run_bass_kernel_spmd
from concourse.masks import make_identity

# Problem constants
T = 4096
DIM = 1024
H = 8
HD = 128
WINDOW = 512
ATTN_SCALE = 0.12
ROPE_BASE = 1024.0
EPS = 1e-6

# Sharding
S = 2          # sequence halves
G = 4          # head groups (2 heads each)
HPC = H // G   # heads per core = 2
TC = T // S    # own rows per core = 2048
TK = TC + WINDOW  # k/v rows incl. halo = 2560
NQT = TC // 128   # q tiles per head = 16
NKC = TK // 128   # k chunks = 20
NPR = TC // 256   # q pairs per head = 8
PW = 256 + WINDOW  # pair window = 768
NPC = PW // 128    # chunks per pair window = 6
EW = 3 * HPC * HD  # fused qkv width per core = 768

F32 = mybir.dt.float32
F32R = mybir.dt.float32r

AOP = mybir.AluOpType
AF = mybir.ActivationFunctionType


def build_kernel():
    nc = bacc.Bacc()

    # Per-core DRAM I/O (shapes identical across cores; data differs).
    xT = nc.declare_dram_parameter("xT", [DIM, TK], F32, isOutput=False)
    wqkvT = nc.declare_dram_parameter("wqkvT", [DIM, EW], F32, isOutput=False)
    woT = nc.declare_dram_parameter("woT", [HPC * HD, DIM], F32, isOutput=False)
    ve = nc.declare_dram_parameter("ve", [TK, HPC * HD], F32, isOutput=False)
    cosT = nc.declare_dram_parameter("cosT", [TK, 32], F32, isOutput=False)
    sinT = nc.declare_dram_parameter("sinT", [TK, 32], F32, isOutput=False)
    lam = nc.declare_dram_parameter("lam", [128, 4], F32, isOutput=False)
    padcnt = nc.declare_dram_parameter("padcnt", [TC], F32, isOutput=False)
    outT = nc.declare_dram_parameter("outT", [DIM, TC], F32, isOutput=True)

    with tile.TileContext(nc) as tc:
        _trace_body(nc, tc, xT, wqkvT, woT, ve, cosT, sinT, lam, padcnt, outT)

    nc.compile()
    return nc


def _trace_body(nc, tc, xT, wqkvT, woT, ve, cosT, sinT, lam, padcnt, outT):
    import contextlib

    ctx = contextlib.ExitStack()
    with ctx:
        const = ctx.enter_context(tc.tile_pool(name="const", bufs=1))
        persist = ctx.enter_context(tc.tile_pool(name="persist", bufs=1))

        # ---- constants needed by phase A (w split per d-chunk so the
        # first projection matmuls can start early) ----
        w_sb = const.tile([128, 8, EW], F32R)  # wqkvT as [dpart, dchunk, e]
        wq_r = wqkvT.rearrange("(a p) e -> p a e", p=128).bitcast(F32R)
        for dch in range(8):
            nc.sync.dma_start(out=w_sb[:, dch, :], in_=wq_r[:, dch, :])
        cos_sb = const.tile([128, NKC, 32], F32)
        nc.sync.dma_start(out=cos_sb, in_=cosT.rearrange("(a p) f -> p a f", p=128))
        sin_sb = const.tile([128, NKC, 32], F32)
        nc.sync.dma_start(out=sin_sb, in_=sinT.rearrange("(a p) f -> p a f", p=128))
        lam_sb = const.tile([128, 4], F32)
        nc.sync.dma_start(out=lam_sb, in_=lam[:])

        identity = const.tile([128, 128], F32R)
        idf = const.tile([128, 128], F32)
        make_identity(nc, idf)
        nc.vector.tensor_copy(out=identity, in_=idf)

        eps_sb = const.tile([128, 1], F32)
        nc.vector.memset(eps_sb, EPS)

        # ---- B/C constant tiles (DMAs deferred until after phase A) ----
        wo_sb = const.tile([128, HPC, DIM], F32R)  # woT as [ddpart, head, e]
        pad_r = const.tile([1, TC], F32)

        onescf = const.tile([128, 1], F32)
        nc.vector.memset(onescf, 1.0)
        ones_col = const.tile([128, 1], F32R)
        nc.vector.tensor_copy(out=ones_col, in_=onescf)

        # Band masks in [kj, ci, qi] orientation for pair-window chunks.
        # Chunk c of a pair window is valid iff qi+1 <= 128c + kj <= qi+512.
        # Chunks 2,3 are always fully valid; 0,1 need the lower bound and
        # 4,5 the upper bound.
        maskA = const.tile([128, 2, 256], F32)  # chunks 0,1
        nc.gpsimd.memset(maskA, 1.0)
        nc.gpsimd.affine_select(
            out=maskA, in_=maskA, compare_op=AOP.is_ge, fill=0.0,
            base=-1, channel_multiplier=1, pattern=[[128, 2], [-1, 256]],
        )
        maskB = const.tile([128, 2, 256], F32)  # chunks 4,5
        nc.gpsimd.memset(maskB, 1.0)
        nc.gpsimd.affine_select(
            out=maskB, in_=maskB, compare_op=AOP.is_ge, fill=0.0,
            base=0, channel_multiplier=-1, pattern=[[-128, 2], [1, 256]],
        )

        # ---- persistent activations ----
        # qT/kT: [dd, t] per head; v: [t(kj) part, chunk, dd]; yT: [dd, t].
        qT = [persist.tile([128, TC], F32R, name=f"qT{h}") for h in range(HPC)]
        kT = [persist.tile([128, TK], F32R, name=f"kT{h}") for h in range(HPC)]
        vbf = [persist.tile([128, NKC, HD], F32R, name=f"vbf{h}") for h in range(HPC)]
        yT = [persist.tile([128, TC], F32R, name=f"yT{h}") for h in range(HPC)]

        # ================= Phase A: QKV projection + norm/rope =================
        with (
            tc.tile_pool(name="xt_pool", bufs=2) as xt_pool,
            tc.tile_pool(name="ve_pool", bufs=2) as ve_pool,
            tc.tile_pool(name="stageA", bufs=4) as stageA,
            tc.tile_pool(name="smallA", bufs=6) as smallA,
            tc.tile_pool(name="proj_psum", bufs=3, space="PSUM") as proj_psum,
            tc.tile_pool(name="tp_psum", bufs=2, space="PSUM") as tp_psum,
        ):
            xT_r = xT.rearrange("(a p) t -> p a t", p=128)  # [128, 8, TK]
            ve_r = ve.rearrange("(a p) d -> p a d", p=128)  # [128, 20, 256]
            TB = 512  # t rows per x block load
            for tb in range(TK // TB):
                xt = xt_pool.tile([128, 8, TB], F32R)
                # split into two DMAs so compute can start on the first half
                xsrc = xT_r[:, :, tb * TB:(tb + 1) * TB].bitcast(F32R)
                nc.sync.dma_start(out=xt[:, 0:4, :], in_=xsrc[:, 0:4, :])
                nc.sync.dma_start(out=xt[:, 4:8, :], in_=xsrc[:, 4:8, :])
                vet = ve_pool.tile([128, 4, HPC * HD], F32)
                nc.sync.dma_start(out=vet, in_=ve_r[:, tb * 4:(tb + 1) * 4, :])
                for tsub in range(TB // 128):
                    c = tb * (TB // 128) + tsub  # t-chunk index, 0..19
                    psum = proj_psum.tile([128, EW], F32)
                    for dch in range(8):
                        lhsT = xt[:, dch, tsub * 128:(tsub + 1) * 128]
                        if c >= 4:
                            nc.tensor.matmul(
                                psum[:, 0:512], lhsT, w_sb[:, dch, 0:512],
                                start=(dch == 0), stop=(dch == 7),
                            )
                        else:  # halo rows need only k,v
                            nc.tensor.matmul(
                                psum[:, 256:512], lhsT, w_sb[:, dch, 256:512],
                                start=(dch == 0), stop=(dch == 7),
                            )
                        nc.tensor.matmul(
                            psum[:, 512:EW], lhsT, w_sb[:, dch, 512:EW],
                            start=(dch == 0), stop=(dch == 7),
                        )
                    # psum segments: q0 q1 k0 k1 v0 v1, each [128, 128]
                    psum6 = psum.rearrange("p (s d) -> p s d", s=6)

                    # RMS-norm scales (halo chunks skip the q segments).
                    # Square on ACT with fused per-segment row-sum accumulation.
                    s0 = 0 if c >= 4 else 2
                    sq = stageA.tile([128, 6, HD], F32)
                    ssum = smallA.tile([128, 6], F32)
                    for sg in range(s0, 6):
                        nc.scalar.activation(
                            sq[:, sg, :], psum6[:, sg, :], AF.Square,
                            accum_out=ssum[:, sg:sg + 1],
                        )
                    # rms for q,k (eps bias) and v (lam0 folded via scale/bias)
                    rms = smallA.tile([128, 6], F32)
                    nc.scalar.activation(rms[:, s0:4], ssum[:, s0:4], AF.Sqrt,
                                         bias=eps_sb, scale=1.0 / HD)
                    nc.scalar.activation(rms[:, 4:6], ssum[:, 4:6], AF.Sqrt,
                                         bias=lam_sb[:, 3:4],
                                         scale=lam_sb[:, 2:3])
                    rs = smallA.tile([128, 6], F32)
                    nc.vector.reciprocal(rs[:, s0:6], rms[:, s0:6])

                    # normalize segments in one DVE op -> staging (f32r)
                    st6 = stageA.tile([128, 6, HD], F32R)
                    nc.vector.tensor_tensor(
                        out=st6[:, s0:6, :], in0=psum6[:, s0:6, :],
                        in1=rs[:, s0:6, None].to_broadcast([128, 6 - s0, HD]),
                        op=AOP.mult,
                    )
                    st6f = st6.bitcast(F32)

                    # v = lam1 * ve + v_normed (gpsimd; all-SBUF).
                    # Pool has no TensorScalarPtr, so use two tensor_tensor
                    # ops with a broadcast lam1 operand.
                    vel = stageA.tile([128, 2, HD], F32, name="vel")
                    nc.gpsimd.tensor_tensor(
                        out=vel, in0=vet[:, tsub, :].rearrange("p (h d) -> p h d", h=2),
                        in1=lam_sb[:, 1:2, None].to_broadcast([128, 2, HD]),
                        op=AOP.mult,
                    )
                    for h in range(HPC):
                        nc.vector.tensor_tensor(
                            out=vbf[h][:, c, :], in0=vel[:, h, :],
                            in1=st6f[:, 4 + h, :], op=AOP.add,
                        )

                    # rope on q,k (dims 0:32 rotate with dims 64:96); gpsimd
                    nseg = 4 - s0
                    cs = cos_sb[:, c:c + 1, :].to_broadcast([128, nseg, 32])
                    sn = sin_sb[:, c:c + 1, :].to_broadcast([128, nseg, 32])
                    x1 = st6f[:, s0:4, 0:32]
                    x2 = st6f[:, s0:4, 64:96]
                    t1 = stageA.tile([128, 4, 32], F32)
                    t2 = stageA.tile([128, 4, 32], F32)
                    t3 = stageA.tile([128, 4, 32], F32)
                    t4 = stageA.tile([128, 4, 32], F32)
                    nc.vector.tensor_tensor(out=t1[:, s0:4, :], in0=x1, in1=cs, op=AOP.mult)
                    nc.vector.tensor_tensor(out=t2[:, s0:4, :], in0=x2, in1=sn, op=AOP.mult)
                    nc.gpsimd.tensor_tensor(out=t3[:, s0:4, :], in0=x1, in1=sn, op=AOP.mult)
                    nc.gpsimd.tensor_tensor(out=t4[:, s0:4, :], in0=x2, in1=cs, op=AOP.mult)
                    nc.vector.tensor_add(st6[:, s0:4, 0:32], t1[:, s0:4, :], t2[:, s0:4, :])
                    nc.vector.tensor_sub(st6[:, s0:4, 64:96], t4[:, s0:4, :], t3[:, s0:4, :])

                    # transpose q,k into [dd, t] persistent buffers (f32r)
                    for h in range(HPC):
                        if c >= 4:  # q exists only for own rows
                            tq = tp_psum.tile([128, 128], F32R, name="tq", tag="tp")
                            nc.tensor.transpose(tq, st6[:, h, :], identity)
                            nc.scalar.copy(
                                out=qT[h][:, (c - 4) * 128:(c - 3) * 128], in_=tq)
                        tk = tp_psum.tile([128, 128], F32R, name="tk", tag="tp")
                        nc.tensor.transpose(tk, st6[:, 2 + h, :], identity)
                        nc.vector.tensor_copy(out=kT[h][:, c * 128:(c + 1) * 128],
                                              in_=tk)

        nc.sync.dma_start(
            out=wo_sb, in_=woT.rearrange("(a p) e -> p a e", p=128).bitcast(F32R))
        nc.sync.dma_start(out=pad_r, in_=padcnt.rearrange("(a t) -> a t", a=1))

        # ====== Phase B+C: banded attention with interleaved out-projection ===
        with (
            tc.tile_pool(name="pm_pool", bufs=3) as pm_pool,
            tc.tile_pool(name="smallB", bufs=4) as smallB,
            tc.tile_pool(name="o_out", bufs=3) as o_out,
            tc.tile_pool(name="sc_psum", bufs=3, space="PSUM") as sc_psum,
            tc.tile_pool(name="sum_psum", bufs=1, space="PSUM") as sum_psum,
            tc.tile_pool(name="y_psum", bufs=2, space="PSUM") as y_psum,
            tc.tile_pool(name="o_psum", bufs=2, space="PSUM") as o_psum,
        ):
            def oproj_window(tw):
                # out[:, 512tw:512tw+512] = sum_h woT_h^T @ yT_h window
                for ec in range(8):
                    ops = o_psum.tile([128, 512], F32, name="ops")
                    for h in range(HPC):
                        nc.tensor.matmul(
                            ops,
                            wo_sb[:, h, ec * 128:(ec + 1) * 128],
                            yT[h][:, tw * 512:(tw + 1) * 512],
                            start=(h == 0), stop=(h == HPC - 1),
                            skip_group_check=True,
                        )
                    ot = o_out.tile([128, 512], F32, name="ot")
                    if ec % 2 == 0:
                        nc.scalar.copy(out=ot, in_=ops)
                    else:
                        nc.vector.tensor_copy(out=ot, in_=ops)
                    nc.sync.dma_start(
                        out=outT[ec * 128:(ec + 1) * 128,
                                 tw * 512:(tw + 1) * 512],
                        in_=ot,
                    )

            for pr in range(NPR):
                for h in range(HPC):
                    qs = qT[h][:, pr * 256:(pr + 1) * 256]
                    pm = pm_pool.tile([128, NPC, 256], F32R)
                    sums = sum_psum.tile([1, 256], F32, name="sums")
                    yps = y_psum.tile([128, 256], F32, name="yps")
                    # masked chunk pairs first so the final accumulation
                    # tail has no Pool mask op on its critical path
                    for i, wp in enumerate((0, 2, 1)):  # chunk pairs
                        sc = sc_psum.tile([128, 2, 256], F32, name="sc", tag="sc")
                        for j in range(2):
                            wc = 2 * wp + j
                            nc.tensor.matmul(
                                sc[:, j, :],
                                kT[h][:, (2 * pr + wc) * 128:(2 * pr + wc + 1) * 128],
                                qs, start=True, stop=True, skip_group_check=True,
                            )
                        nc.scalar.activation(pm[:, 2 * wp:2 * wp + 2, :], sc,
                                             AF.Exp, scale=ATTN_SCALE)
                        if wp == 0:
                            nc.vector.tensor_tensor(
                                out=pm[:, 0:2, :], in0=pm[:, 0:2, :].bitcast(F32),
                                in1=maskA, op=AOP.mult)
                        elif wp == 2:
                            nc.vector.tensor_tensor(
                                out=pm[:, 4:6, :], in0=pm[:, 4:6, :].bitcast(F32),
                                in1=maskB, op=AOP.mult)
                        for j in range(2):
                            wc = 2 * wp + j
                            nc.tensor.matmul(
                                sums, ones_col, pm[:, wc, :],
                                start=(i == 0 and j == 0),
                                stop=(i == 2 and j == 1),
                                skip_group_check=True,
                            )
                            nc.tensor.matmul(
                                yps, vbf[h][:, 2 * pr + wc, :], pm[:, wc, :],
                                start=(i == 0 and j == 0),
                                stop=(i == 2 and j == 1),
                                skip_group_check=True,
                            )
                    sums2 = smallB.tile([1, 256], F32)
                    nc.vector.tensor_sub(sums2, sums,
                                         pad_r[:, pr * 256:(pr + 1) * 256])
                    recip = smallB.tile([1, 256], F32)
                    nc.vector.reciprocal(recip, sums2)
                    # broadcast 1/sum across partitions on the idle Pool engine
                    bc_sb = smallB.tile([128, 256], F32, name="bc_sb")
                    nc.gpsimd.partition_broadcast(bc_sb, recip)
                    # evacuate with the 1/sum normalization fused (cast f32r)
                    nc.vector.tensor_tensor(
                        out=yT[h][:, pr * 256:(pr + 1) * 256],
                        in0=yps, in1=bc_sb, op=AOP.mult)
                if pr % 2 == 1:
                    oproj_window(pr // 2)


_NC_CACHE = None


def _get_nc():
    global _NC_CACHE
    if _NC_CACHE is None:
        _NC_CACHE = build_kernel()
    return _NC_CACHE


def _rope_tables(positions):
    keep = HD // 4
    active = (1.0 / ROPE_BASE) ** np.linspace(0.0, 1.0, keep, dtype=np.float32)
    theta = positions[:, None].astype(np.float32) * active[None, :]  # [n, 32]
    return np.cos(theta).astype(np.float32), np.sin(theta).astype(np.float32)


def make_in_maps(x, ve, lambdas, qkvo_w):
    """Build the 8 per-core input maps from full inputs (host-side sharding)."""
    x2 = x.reshape(T, DIM)
    ve2 = ve.reshape(T, DIM)
    qw, kw, vw, ow = qkvo_w[0], qkvo_w[1], qkvo_w[2], qkvo_w[3]

    in_maps = []
    for c in range(8):
        s, g = divmod(c, G)
        h0, h1 = HPC * g, HPC * g + 1
        lo = TC * s - WINDOW  # first k/v row (may be negative -> zero pad)
        hi = TC * s + TC

        # xT slice with zero pad
        xs = np.zeros((TK, DIM), np.float32)
        src_lo = max(lo, 0)
        xs[src_lo - lo:, :] = x2[src_lo:hi, :]
        xTc = np.ascontiguousarray(xs.T)

        # fused qkv weight, transposed: cols = q0 q1 k0 k1 v0 v1
        wcols = []
        for wmat in (qw, kw, vw):
            for h in (h0, h1):
                wcols.append(wmat[h * HD:(h + 1) * HD, :].T)
        wqkvT = np.ascontiguousarray(np.concatenate(wcols, axis=1))

        woT = np.ascontiguousarray(ow[:, h0 * HD:(h1 + 1) * HD].T)

        ves = np.zeros((TK, HPC * HD), np.float32)
        ves[src_lo - lo:, :] = ve2[src_lo:hi, h0 * HD:(h1 + 1) * HD]

        pos = np.clip(np.arange(lo, hi), 0, None)
        cosT, sinT = _rope_tables(pos)

        l0, l1 = float(lambdas[0]), float(lambdas[1])
        lam_row = np.array([l0, l1, 1.0 / (HD * l0 * l0), EPS / (l0 * l0)],
                           np.float32)
        lam = np.tile(lam_row.reshape(1, 4), (128, 1)).astype(np.float32)

        pc = np.zeros(TC, np.float32)
        if s == 0:
            i = np.arange(TC)
            pc = np.maximum(0.0, WINDOW - 1.0 - i).astype(np.float32)

        in_maps.append({
            "xT": xTc, "wqkvT": wqkvT, "woT": woT, "ve": ves,
            "cosT": cosT, "sinT": sinT, "lam": lam, "padcnt": pc,
        })
    return in_maps


def kernel(x, ve, lambdas, qkvo_w, window):
    assert int(window) == WINDOW
    x = np.asarray(x, np.float32)
    ve = np.asarray(ve, np.float32)
    lambdas = np.asarray(lambdas, np.float32)
    qkvo_w = np.asarray(qkvo_w, np.float32)

    nc = _get_nc()
    in_maps = make_in_maps(x, ve, lambdas, qkvo_w)
    res = run_bass_kernel_spmd(nc, in_maps, core_ids=list(range(8)))

    outT_full = np.zeros((DIM, T), np.float32)
    for c in range(8):
        s = c // G
        outT_full[:, TC * s:TC * (s + 1)] += res.results[c]["outT"]
    return np.ascontiguousarray(outT_full.T).reshape(1, T, DIM)


if __name__ == "__main__":
    nc = _get_nc()
    print("kernel built ok")


# revision 21
# speedup vs baseline: 1.1041x; 1.1041x over previous
"""Trainium2 Bass kernel for sliding-window causal self-attention.

Reference computation (B=1, T=4096, H=8 heads, head_dim=128, DIM=1024):
  qkv = x @ w_qkv.T; q,k = rms_norm -> rope; v = lam0*rms_norm(v) + lam1*ve
  scores = (q k^T) * 0.12 with sliding-window causal mask (0 <= i-j < 512)
  y = softmax(scores) @ v;  out = y @ o_w.T

Sharding over 8 cores: 2 sequence halves (S) x 4 head-pair groups (G).
Core c = 4*s + g handles t in [2048s, 2048(s+1)) for heads {2g, 2g+1}.
Each core reads its x rows plus a 512-row halo of preceding rows (for k/v),
computes its partial output projection over its 2 heads, and the host sums
the 4 partials per half and concatenates the halves. No on-chip collectives.

Attention uses a transposeless [kj, qi] scores layout: q-tiles are processed
in pairs (256 queries, 768-key window, 6 key chunks) so every matmul has a
moving free dim >= 256, which is required for full-rate float32r matmuls.
Softmax runs without max-subtraction (scores are bounded by 0.12*128), the
kj-sum is done on the PE with a ones vector, and the reciprocal is broadcast
across partitions with a rank-1 matmul. The output projection is interleaved
into the attention loop so its DMA overlaps compute. Elementwise work is
spread across DVE / ScalarE / GpSimd to keep all engines busy.
"""

import sys

sys.path.insert(0, "/opt/trn_rl_repo")

import numpy as np

import concourse.bass as bass
import concourse.mybir as mybir
import concourse.tile as tile
from concourse import bacc
from concourse.bass_utils import run_bass_kernel_spmd
from concourse.masks import make_identity

# Problem constants
T = 4096
DIM = 1024
H = 8
HD = 128
WINDOW = 512
ATTN_SCALE = 0.12
ROPE_BASE = 1024.0
EPS = 1e-6

# Sharding
S = 2          # sequence halves
G = 4          # head groups (2 heads each)
HPC = H // G   # heads per core = 2
TC = T // S    # own rows per core = 2048
TK = TC + WINDOW  # k/v rows incl. halo = 2560
NQT = TC // 128   # q tiles per head = 16
NKC = TK // 128   # k chunks = 20
NPR = TC // 256   # q pairs per head = 8
PW = 256 + WINDOW  # pair window = 768
NPC = PW // 128    # chunks per pair window = 6
EW = 3 * HPC * HD  # fused qkv width per core = 768

F32 = mybir.dt.float32
F32R = mybir.dt.float32r

AOP = mybir.AluOpType
AF = mybir.ActivationFunctionType


def build_kernel():
    nc = bacc.Bacc()

    # Per-core DRAM I/O (shapes identical across cores; data differs).
    xT = nc.declare_dram_parameter("xT", [DIM, TK], F32, isOutput=False)
    wqkvT = nc.declare_dram_parameter("wqkvT", [DIM, EW], F32, isOutput=False)
    woT = nc.declare_dram_parameter("woT", [HPC * HD, DIM], F32, isOutput=False)
    ve = nc.declare_dram_parameter("ve", [TK, HPC * HD], F32, isOutput=False)
    cosT = nc.declare_dram_parameter("cosT", [TK, 32], F32, isOutput=False)
    sinT = nc.declare_dram_parameter("sinT", [TK, 32], F32, isOutput=False)
    lam = nc.declare_dram_parameter("lam", [128, 4], F32, isOutput=False)
    padcnt = nc.declare_dram_parameter("padcnt", [TC], F32, isOutput=False)
    outT = nc.declare_dram_parameter("outT", [DIM, TC], F32, isOutput=True)

    with tile.TileContext(nc) as tc:
        _trace_body(nc, tc, xT, wqkvT, woT, ve, cosT, sinT, lam, padcnt, outT)

    nc.compile()
    return nc


def _trace_body(nc, tc, xT, wqkvT, woT, ve, cosT, sinT, lam, padcnt, outT):
    import contextlib

    ctx = contextlib.ExitStack()
    with ctx:
        const = ctx.enter_context(tc.tile_pool(name="const", bufs=1))
        persist = ctx.enter_context(tc.tile_pool(name="persist", bufs=1))

        # ---- constants needed by phase A (w split per d-chunk so the
        # first projection matmuls can start early) ----
        w_sb = const.tile([128, 8, EW], F32R)  # wqkvT as [dpart, dchunk, e]
        wq_r = wqkvT.rearrange("(a p) e -> p a e", p=128).bitcast(F32R)
        for dch in range(8):
            nc.sync.dma_start(out=w_sb[:, dch, :], in_=wq_r[:, dch, :])
        cos_sb = const.tile([128, NKC, 32], F32)
        nc.sync.dma_start(out=cos_sb, in_=cosT.rearrange("(a p) f -> p a f", p=128))
        sin_sb = const.tile([128, NKC, 32], F32)
        nc.sync.dma_start(out=sin_sb, in_=sinT.rearrange("(a p) f -> p a f", p=128))
        lam_sb = const.tile([128, 4], F32)
        nc.sync.dma_start(out=lam_sb, in_=lam[:])

        identity = const.tile([128, 128], F32R)
        idf = const.tile([128, 128], F32)
        make_identity(nc, idf)
        nc.vector.tensor_copy(out=identity, in_=idf)

        eps_sb = const.tile([128, 1], F32)
        nc.vector.memset(eps_sb, EPS)

        # ---- B/C constant tiles (DMAs deferred until after phase A) ----
        wo_sb = const.tile([128, HPC, DIM], F32R)  # woT as [ddpart, head, e]
        pad_r = const.tile([1, TC], F32)

        onescf = const.tile([128, 1], F32)
        nc.vector.memset(onescf, 1.0)
        ones_col = const.tile([128, 1], F32R)
        nc.vector.tensor_copy(out=ones_col, in_=onescf)

        # Band masks in [kj, ci, qi] orientation for pair-window chunks.
        # Chunk c of a pair window is valid iff qi+1 <= 128c + kj <= qi+512.
        # Chunks 2,3 are always fully valid; 0,1 need the lower bound and
        # 4,5 the upper bound.
        maskA = const.tile([128, 2, 256], F32)  # chunks 0,1
        nc.gpsimd.memset(maskA, 1.0)
        nc.gpsimd.affine_select(
            out=maskA, in_=maskA, compare_op=AOP.is_ge, fill=0.0,
            base=-1, channel_multiplier=1, pattern=[[128, 2], [-1, 256]],
        )
        maskB = const.tile([128, 2, 256], F32)  # chunks 4,5
        nc.gpsimd.memset(maskB, 1.0)
        nc.gpsimd.affine_select(
            out=maskB, in_=maskB, compare_op=AOP.is_ge, fill=0.0,
            base=0, channel_multiplier=-1, pattern=[[-128, 2], [1, 256]],
        )

        # ---- persistent activations ----
        # qT/kT: [dd, t] per head; v: [t(kj) part, chunk, dd]; yT: [dd, t].
        qT = [persist.tile([128, TC], F32R, name=f"qT{h}") for h in range(HPC)]
        kT = [persist.tile([128, TK], F32R, name=f"kT{h}") for h in range(HPC)]
        vbf = [persist.tile([128, NKC, HD], F32R, name=f"vbf{h}") for h in range(HPC)]
        yT = [persist.tile([128, TC], F32R, name=f"yT{h}") for h in range(HPC)]

        # ================= Phase A: QKV projection + norm/rope =================
        with (
            tc.tile_pool(name="xt_pool", bufs=2) as xt_pool,
            tc.tile_pool(name="ve_pool", bufs=2) as ve_pool,
            tc.tile_pool(name="stageA", bufs=4) as stageA,
            tc.tile_pool(name="smallA", bufs=8) as smallA,
            tc.tile_pool(name="proj_psum", bufs=3, space="PSUM") as proj_psum,
            tc.tile_pool(name="tp_psum", bufs=2, space="PSUM") as tp_psum,
        ):
            xT_r = xT.rearrange("(a p) t -> p a t", p=128)  # [128, 8, TK]
            ve_r = ve.rearrange("(a p) d -> p a d", p=128)  # [128, 20, 256]
            TB = 512  # t rows per x block load
            for tb in range(TK // TB):
                xt = xt_pool.tile([128, 8, TB], F32R)
                # split into two DMAs so compute can start on the first half
                xsrc = xT_r[:, :, tb * TB:(tb + 1) * TB].bitcast(F32R)
                nc.sync.dma_start(out=xt[:, 0:4, :], in_=xsrc[:, 0:4, :])
                nc.sync.dma_start(out=xt[:, 4:8, :], in_=xsrc[:, 4:8, :])
                vet = ve_pool.tile([128, 4, HPC * HD], F32)
                nc.sync.dma_start(out=vet, in_=ve_r[:, tb * 4:(tb + 1) * 4, :])
                for tsub in range(TB // 128):
                    c = tb * (TB // 128) + tsub  # t-chunk index, 0..19
                    psum = proj_psum.tile([128, EW], F32)
                    for dch in range(8):
                        lhsT = xt[:, dch, tsub * 128:(tsub + 1) * 128]
                        if c >= 4:
                            nc.tensor.matmul(
                                psum[:, 0:512], lhsT, w_sb[:, dch, 0:512],
                                start=(dch == 0), stop=(dch == 7),
                            )
                        else:  # halo rows need only k,v
                            nc.tensor.matmul(
                                psum[:, 256:512], lhsT, w_sb[:, dch, 256:512],
                                start=(dch == 0), stop=(dch == 7),
                            )
                        nc.tensor.matmul(
                            psum[:, 512:EW], lhsT, w_sb[:, dch, 512:EW],
                            start=(dch == 0), stop=(dch == 7),
                        )
                    # psum segments: q0 q1 k0 k1 v0 v1, each [128, 128]
                    psum6 = psum.rearrange("p (s d) -> p s d", s=6)

                    # RMS-norm scales (halo chunks skip the q segments).
                    # Square on ACT with fused per-segment row-sum accumulation.
                    s0 = 0 if c >= 4 else 2
                    sq = stageA.tile([128, 6, HD], F32)
                    ssum = smallA.tile([128, 6], F32)
                    for sg in range(s0, 6):
                        nc.scalar.activation(
                            sq[:, sg, :], psum6[:, sg, :], AF.Square,
                            accum_out=ssum[:, sg:sg + 1],
                        )
                    # rms for q,k (eps bias) and v (lam0 folded via scale/bias)
                    rms = smallA.tile([128, 6], F32)
                    nc.scalar.activation(rms[:, s0:4], ssum[:, s0:4], AF.Sqrt,
                                         bias=eps_sb, scale=1.0 / HD)
                    nc.scalar.activation(rms[:, 4:6], ssum[:, 4:6], AF.Sqrt,
                                         bias=lam_sb[:, 3:4],
                                         scale=lam_sb[:, 2:3])
                    rs = smallA.tile([128, 6], F32)
                    nc.vector.reciprocal(rs[:, s0:6], rms[:, s0:6])

                    # normalize segments in one DVE op -> staging (f32r)
                    st6 = stageA.tile([128, 6, HD], F32R)
                    nc.vector.tensor_tensor(
                        out=st6[:, s0:6, :], in0=psum6[:, s0:6, :],
                        in1=rs[:, s0:6, None].to_broadcast([128, 6 - s0, HD]),
                        op=AOP.mult,
                    )
                    st6f = st6.bitcast(F32)

                    # v = lam1 * ve + v_normed (gpsimd; all-SBUF).
                    # Pool has no TensorScalarPtr, so use two tensor_tensor
                    # ops with a broadcast lam1 operand.
                    vel = stageA.tile([128, 2, HD], F32, name="vel")
                    nc.gpsimd.tensor_tensor(
                        out=vel, in0=vet[:, tsub, :].rearrange("p (h d) -> p h d", h=2),
                        in1=lam_sb[:, 1:2, None].to_broadcast([128, 2, HD]),
                        op=AOP.mult,
                    )
                    for h in range(HPC):
                        nc.vector.tensor_tensor(
                            out=vbf[h][:, c, :], in0=vel[:, h, :],
                            in1=st6f[:, 4 + h, :], op=AOP.add,
                        )

                    # rope on q,k (dims 0:32 rotate with dims 64:96); gpsimd
                    nseg = 4 - s0
                    cs = cos_sb[:, c:c + 1, :].to_broadcast([128, nseg, 32])
                    sn = sin_sb[:, c:c + 1, :].to_broadcast([128, nseg, 32])
                    x1 = st6f[:, s0:4, 0:32]
                    x2 = st6f[:, s0:4, 64:96]
                    t1 = stageA.tile([128, 4, 32], F32)
                    t2 = stageA.tile([128, 4, 32], F32)
                    t3 = stageA.tile([128, 4, 32], F32)
                    t4 = stageA.tile([128, 4, 32], F32)
                    nc.vector.tensor_tensor(out=t1[:, s0:4, :], in0=x1, in1=cs, op=AOP.mult)
                    nc.vector.tensor_tensor(out=t2[:, s0:4, :], in0=x2, in1=sn, op=AOP.mult)
                    nc.gpsimd.tensor_tensor(out=t3[:, s0:4, :], in0=x1, in1=sn, op=AOP.mult)
                    nc.gpsimd.tensor_tensor(out=t4[:, s0:4, :], in0=x2, in1=cs, op=AOP.mult)
                    nc.vector.tensor_add(st6[:, s0:4, 0:32], t1[:, s0:4, :], t2[:, s0:4, :])
                    nc.vector.tensor_sub(st6[:, s0:4, 64:96], t4[:, s0:4, :], t3[:, s0:4, :])

                    # transpose q,k into [dd, t] persistent buffers (f32r)
                    for h in range(HPC):
                        if c >= 4:  # q exists only for own rows
                            tq = tp_psum.tile([128, 128], F32R, name="tq", tag="tp")
                            nc.tensor.transpose(tq, st6[:, h, :], identity)
                            nc.scalar.copy(
                                out=qT[h][:, (c - 4) * 128:(c - 3) * 128], in_=tq)
                        tk = tp_psum.tile([128, 128], F32R, name="tk", tag="tp")
                        nc.tensor.transpose(tk, st6[:, 2 + h, :], identity)
                        nc.vector.tensor_copy(out=kT[h][:, c * 128:(c + 1) * 128],
                                              in_=tk)

        nc.sync.dma_start(
            out=wo_sb, in_=woT.rearrange("(a p) e -> p a e", p=128).bitcast(F32R))
        nc.sync.dma_start(out=pad_r, in_=padcnt.rearrange("(a t) -> a t", a=1))

        # ====== Phase B+C: banded attention with interleaved out-projection ===
        with (
            tc.tile_pool(name="pm_pool", bufs=3) as pm_pool,
            tc.tile_pool(name="smallB", bufs=8) as smallB,
            tc.tile_pool(name="o_out", bufs=4) as o_out,
            tc.tile_pool(name="sc_psum", bufs=3, space="PSUM") as sc_psum,
            tc.tile_pool(name="sum_psum", bufs=1, space="PSUM") as sum_psum,
            tc.tile_pool(name="y_psum", bufs=2, space="PSUM") as y_psum,
            tc.tile_pool(name="o_psum", bufs=2, space="PSUM") as o_psum,
        ):
            def oproj_window(tw):
                # out[:, 512tw:512tw+512] = sum_h woT_h^T @ yT_h window
                for ec in range(8):
                    ops = o_psum.tile([128, 512], F32, name="ops")
                    for h in range(HPC):
                        nc.tensor.matmul(
                            ops,
                            wo_sb[:, h, ec * 128:(ec + 1) * 128],
                            yT[h][:, tw * 512:(tw + 1) * 512],
                            start=(h == 0), stop=(h == HPC - 1),
                            skip_group_check=True,
                        )
                    ot = o_out.tile([128, 512], F32, name="ot")
                    if ec % 2 == 0:
                        nc.scalar.copy(out=ot, in_=ops)
                    else:
                        nc.vector.tensor_copy(out=ot, in_=ops)
                    nc.sync.dma_start(
                        out=outT[ec * 128:(ec + 1) * 128,
                                 tw * 512:(tw + 1) * 512],
                        in_=ot,
                    )

            for pr in range(NPR):
                for h in range(HPC):
                    qs = qT[h][:, pr * 256:(pr + 1) * 256]
                    pm = pm_pool.tile([128, NPC, 256], F32R)
                    sums = sum_psum.tile([1, 256], F32, name="sums")
                    yps = y_psum.tile([128, 256], F32, name="yps")
                    # masked chunk pairs first so the final accumulation
                    # tail has no Pool mask op on its critical path
                    for i, wp in enumerate((0, 2, 1)):  # chunk pairs
                        sc = sc_psum.tile([128, 2, 256], F32, name="sc", tag="sc")
                        for j in range(2):
                            wc = 2 * wp + j
                            nc.tensor.matmul(
                                sc[:, j, :],
                                kT[h][:, (2 * pr + wc) * 128:(2 * pr + wc + 1) * 128],
                                qs, start=True, stop=True, skip_group_check=True,
                            )
                        nc.scalar.activation(pm[:, 2 * wp:2 * wp + 2, :], sc,
                                             AF.Exp, scale=ATTN_SCALE)
                        if wp == 0:
                            nc.vector.tensor_tensor(
                                out=pm[:, 0:2, :], in0=pm[:, 0:2, :].bitcast(F32),
                                in1=maskA, op=AOP.mult)
                        elif wp == 2:
                            nc.vector.tensor_tensor(
                                out=pm[:, 4:6, :], in0=pm[:, 4:6, :].bitcast(F32),
                                in1=maskB, op=AOP.mult)
                        for j in range(2):
                            wc = 2 * wp + j
                            nc.tensor.matmul(
                                sums, ones_col, pm[:, wc, :],
                                start=(i == 0 and j == 0),
                                stop=(i == 2 and j == 1),
                                skip_group_check=True,
                            )
                            nc.tensor.matmul(
                                yps, vbf[h][:, 2 * pr + wc, :], pm[:, wc, :],
                                start=(i == 0 and j == 0),
                                stop=(i == 2 and j == 1),
                                skip_group_check=True,
                            )
                    sums2 = smallB.tile([1, 256], F32)
                    nc.vector.tensor_sub(sums2, sums,
                                         pad_r[:, pr * 256:(pr + 1) * 256])
                    recip = smallB.tile([1, 256], F32)
                    nc.vector.reciprocal(recip, sums2)
                    # broadcast 1/sum across partitions on the idle Pool engine
                    bc_sb = smallB.tile([128, 256], F32, name="bc_sb")
                    nc.gpsimd.partition_broadcast(bc_sb, recip)
                    # evacuate with the 1/sum normalization fused (cast f32r)
                    nc.vector.tensor_tensor(
                        out=yT[h][:, pr * 256:(pr + 1) * 256],
                        in0=yps, in1=bc_sb, op=AOP.mult)
                if pr % 2 == 1:
                    oproj_window(pr // 2)


_NC_CACHE = None


def _get_nc():
    global _NC_CACHE
    if _NC_CACHE is None:
        _NC_CACHE = build_kernel()
    return _NC_CACHE


def _rope_tables(positions):
    keep = HD // 4
    active = (1.0 / ROPE_BASE) ** np.linspace(0.0, 1.0, keep, dtype=np.float32)
    theta = positions[:, None].astype(np.float32) * active[None, :]  # [n, 32]
    return np.cos(theta).astype(np.float32), np.sin(theta).astype(np.float32)


def make_in_maps(x, ve, lambdas, qkvo_w):
    """Build the 8 per-core input maps from full inputs (host-side sharding)."""
    x2 = x.reshape(T, DIM)
    ve2 = ve.reshape(T, DIM)
    qw, kw, vw, ow = qkvo_w[0], qkvo_w[1], qkvo_w[2], qkvo_w[3]

    in_maps = []
    for c in range(8):
        s, g = divmod(c, G)
        h0, h1 = HPC * g, HPC * g + 1
        lo = TC * s - WINDOW  # first k/v row (may be negative -> zero pad)
        hi = TC * s + TC

        # xT slice with zero pad
        xs = np.zeros((TK, DIM), np.float32)
        src_lo = max(lo, 0)
        xs[src_lo - lo:, :] = x2[src_lo:hi, :]
        xTc = np.ascontiguousarray(xs.T)

        # fused qkv weight, transposed: cols = q0 q1 k0 k1 v0 v1
        wcols = []
        for wmat in (qw, kw, vw):
            for h in (h0, h1):
                wcols.append(wmat[h * HD:(h + 1) * HD, :].T)
        wqkvT = np.ascontiguousarray(np.concatenate(wcols, axis=1))

        woT = np.ascontiguousarray(ow[:, h0 * HD:(h1 + 1) * HD].T)

        ves = np.zeros((TK, HPC * HD), np.float32)
        ves[src_lo - lo:, :] = ve2[src_lo:hi, h0 * HD:(h1 + 1) * HD]

        pos = np.clip(np.arange(lo, hi), 0, None)
        cosT, sinT = _rope_tables(pos)

        l0, l1 = float(lambdas[0]), float(lambdas[1])
        lam_row = np.array([l0, l1, 1.0 / (HD * l0 * l0), EPS / (l0 * l0)],
                           np.float32)
        lam = np.tile(lam_row.reshape(1, 4), (128, 1)).astype(np.float32)

        pc = np.zeros(TC, np.float32)
        if s == 0:
            i = np.arange(TC)
            pc = np.maximum(0.0, WINDOW - 1.0 - i).astype(np.float32)

        in_maps.append({
            "xT": xTc, "wqkvT": wqkvT, "woT": woT, "ve": ves,
            "cosT": cosT, "sinT": sinT, "lam": lam, "padcnt": pc,
        })
    return in_maps


def kernel(x, ve, lambdas, qkvo_w, window):
    assert int(window) == WINDOW
    x = np.asarray(x, np.float32)
    ve = np.asarray(ve, np.float32)
    lambdas = np.asarray(lambdas, np.float32)
    qkvo_w = np.asarray(qkvo_w, np.float32)

    nc = _get_nc()
    in_maps = make_in_maps(x, ve, lambdas, qkvo_w)
    res = run_bass_kernel_spmd(nc, in_maps, core_ids=list(range(8)))

    outT_full = np.zeros((DIM, T), np.float32)
    for c in range(8):
        s = c // G
        outT_full[:, TC * s:TC * (s + 1)] += res.results[c]["outT"]
    return np.ascontiguousarray(outT_full.T).reshape(1, T, DIM)


if __name__ == "__main__":
    nc = _get_nc()
    print("kernel built ok")


# revision 25
# speedup vs baseline: 1.1661x; 1.0562x over previous
"""Trainium2 Bass kernel for sliding-window causal self-attention.

Reference computation (B=1, T=4096, H=8 heads, head_dim=128, DIM=1024):
  qkv = x @ w_qkv.T; q,k = rms_norm -> rope; v = lam0*rms_norm(v) + lam1*ve
  scores = (q k^T) * 0.12 with sliding-window causal mask (0 <= i-j < 512)
  y = softmax(scores) @ v;  out = y @ o_w.T

Sharding over 8 cores: 2 sequence halves (S) x 4 head-pair groups (G).
Core c = 4*s + g handles t in [2048s, 2048(s+1)) for heads {2g, 2g+1}.
Each core reads its x rows plus a 512-row halo of preceding rows (for k/v),
computes its partial output projection over its 2 heads, and the host sums
the 4 partials per half and concatenates the halves. No on-chip collectives.

Attention uses a transposeless [kj, qi] scores layout: q-tiles are processed
in pairs (256 queries, 768-key window, 6 key chunks) so every matmul has a
moving free dim >= 256, which is required for full-rate float32r matmuls.
Softmax runs without max-subtraction (scores are bounded by 0.12*128), the
kj-sum is done on the PE with a ones vector, and the reciprocal is broadcast
across partitions with a rank-1 matmul. The output projection is interleaved
into the attention loop so its DMA overlaps compute. Elementwise work is
spread across DVE / ScalarE / GpSimd to keep all engines busy.
"""

import sys

sys.path.insert(0, "/opt/trn_rl_repo")

import numpy as np

import concourse.bass as bass
import concourse.mybir as mybir
import concourse.tile as tile
from concourse import bacc
from concourse.bass_utils import run_bass_kernel_spmd
from concourse.masks import make_identity

# Problem constants
T = 4096
DIM = 1024
H = 8
HD = 128
WINDOW = 512
ATTN_SCALE = 0.12
ROPE_BASE = 1024.0
EPS = 1e-6

# Sharding
S = 2          # sequence halves
G = 4          # head groups (2 heads each)
HPC = H // G   # heads per core = 2
TC = T // S    # own rows per core = 2048
TK = TC + WINDOW  # k/v rows incl. halo = 2560
NQT = TC // 128   # q tiles per head = 16
NKC = TK // 128   # k chunks = 20
NPR = TC // 256   # q pairs per head = 8
PW = 256 + WINDOW  # pair window = 768
NPC = PW // 128    # chunks per pair window = 6
EW = 3 * HPC * HD  # fused qkv width per core = 768

F32 = mybir.dt.float32
F32R = mybir.dt.float32r

AOP = mybir.AluOpType
AF = mybir.ActivationFunctionType


def build_kernel():
    nc = bacc.Bacc()

    # Per-core DRAM I/O (shapes identical across cores; data differs).
    xT = nc.declare_dram_parameter("xT", [DIM, TK], F32, isOutput=False)
    wqkvT = nc.declare_dram_parameter("wqkvT", [DIM, EW], F32, isOutput=False)
    woT = nc.declare_dram_parameter("woT", [HPC * HD, DIM], F32, isOutput=False)
    ve = nc.declare_dram_parameter("ve", [TK, HPC * HD], F32, isOutput=False)
    cosT = nc.declare_dram_parameter("cosT", [TK, 32], F32, isOutput=False)
    sinT = nc.declare_dram_parameter("sinT", [TK, 32], F32, isOutput=False)
    lam = nc.declare_dram_parameter("lam", [128, 4], F32, isOutput=False)
    padcnt = nc.declare_dram_parameter("padcnt", [TC], F32, isOutput=False)
    outT = nc.declare_dram_parameter("outT", [DIM, TC], F32, isOutput=True)

    with tile.TileContext(nc) as tc:
        _trace_body(nc, tc, xT, wqkvT, woT, ve, cosT, sinT, lam, padcnt, outT)

    nc.compile()
    return nc


def _trace_body(nc, tc, xT, wqkvT, woT, ve, cosT, sinT, lam, padcnt, outT):
    import contextlib

    ctx = contextlib.ExitStack()
    with ctx:
        const = ctx.enter_context(tc.tile_pool(name="const", bufs=1))
        persist = ctx.enter_context(tc.tile_pool(name="persist", bufs=1))

        # ---- constants needed by phase A (w split per d-chunk so the
        # first projection matmuls can start early) ----
        w_sb = const.tile([128, 8, EW], F32R)  # wqkvT as [dpart, dchunk, e]
        wq_r = wqkvT.rearrange("(a p) e -> p a e", p=128).bitcast(F32R)
        for dch in range(4):
            nc.sync.dma_start(out=w_sb[:, dch, :], in_=wq_r[:, dch, :])
        cos_sb = const.tile([128, NKC, 32], F32)
        sin_sb = const.tile([128, NKC, 32], F32)
        lam_sb = const.tile([128, 4], F32)

        identity = const.tile([128, 128], F32R)
        idf = const.tile([128, 128], F32)
        make_identity(nc, idf)
        nc.vector.tensor_copy(out=identity, in_=idf)

        eps_sb = const.tile([128, 1], F32)
        nc.vector.memset(eps_sb, EPS)

        # ---- B/C constant tiles (DMAs deferred until after phase A) ----
        wo_sb = const.tile([128, HPC, DIM], F32R)  # woT as [ddpart, head, e]
        pad_r = const.tile([1, TC], F32)

        onescf = const.tile([128, 1], F32)
        nc.vector.memset(onescf, 1.0)
        ones_col = const.tile([128, 1], F32R)
        nc.vector.tensor_copy(out=ones_col, in_=onescf)

        # Band masks in [kj, ci, qi] orientation for pair-window chunks.
        # Chunk c of a pair window is valid iff qi+1 <= 128c + kj <= qi+512.
        # Chunks 2,3 are always fully valid; 0,1 need the lower bound and
        # 4,5 the upper bound.
        maskA = const.tile([128, 2, 256], F32)  # chunks 0,1
        nc.gpsimd.memset(maskA, 1.0)
        nc.gpsimd.affine_select(
            out=maskA, in_=maskA, compare_op=AOP.is_ge, fill=0.0,
            base=-1, channel_multiplier=1, pattern=[[128, 2], [-1, 256]],
        )
        maskB = const.tile([128, 2, 256], F32)  # chunks 4,5
        nc.gpsimd.memset(maskB, 1.0)
        nc.gpsimd.affine_select(
            out=maskB, in_=maskB, compare_op=AOP.is_ge, fill=0.0,
            base=0, channel_multiplier=-1, pattern=[[-128, 2], [1, 256]],
        )

        # ---- persistent activations ----
        # qT/kT: [dd, t] per head; v: [t(kj) part, chunk, dd]; yT: [dd, t].
        qT = [persist.tile([128, TC], F32R, name=f"qT{h}") for h in range(HPC)]
        kT = [persist.tile([128, TK], F32R, name=f"kT{h}") for h in range(HPC)]
        vbf = [persist.tile([128, NKC, HD], F32R, name=f"vbf{h}") for h in range(HPC)]
        yT = [persist.tile([128, TC], F32R, name=f"yT{h}") for h in range(HPC)]

        # ================= Phase A: QKV projection + norm/rope =================
        with (
            tc.tile_pool(name="xt_pool", bufs=2) as xt_pool,
            tc.tile_pool(name="ve_pool", bufs=2) as ve_pool,
            tc.tile_pool(name="stageA", bufs=4) as stageA,
            tc.tile_pool(name="smallA", bufs=8) as smallA,
            tc.tile_pool(name="proj_psum", bufs=3, space="PSUM") as proj_psum,
            tc.tile_pool(name="tp_psum", bufs=2, space="PSUM") as tp_psum,
        ):
            xT_r = xT.rearrange("(a p) t -> p a t", p=128)  # [128, 8, TK]
            ve_r = ve.rearrange("(a p) d -> p a d", p=128)  # [128, 20, 256]
            TB = 512  # t rows per x block load
            for tb in range(TK // TB):
                xt = xt_pool.tile([128, 8, TB], F32R)
                # split into two DMAs so compute can start on the first half
                xsrc = xT_r[:, :, tb * TB:(tb + 1) * TB].bitcast(F32R)
                nc.sync.dma_start(out=xt[:, 0:4, :], in_=xsrc[:, 0:4, :])
                nc.sync.dma_start(out=xt[:, 4:8, :], in_=xsrc[:, 4:8, :])
                vet = ve_pool.tile([128, 4, HPC * HD], F32)
                nc.sync.dma_start(out=vet, in_=ve_r[:, tb * 4:(tb + 1) * 4, :])
                if tb == 0:
                    # bulk loads deferred behind the first x block
                    for dch in range(4, 8):
                        nc.sync.dma_start(out=w_sb[:, dch, :], in_=wq_r[:, dch, :])
                    nc.sync.dma_start(
                        out=cos_sb, in_=cosT.rearrange("(a p) f -> p a f", p=128))
                    nc.sync.dma_start(
                        out=sin_sb, in_=sinT.rearrange("(a p) f -> p a f", p=128))
                    nc.sync.dma_start(out=lam_sb, in_=lam[:])
                for tsub in range(TB // 128):
                    c = tb * (TB // 128) + tsub  # t-chunk index, 0..19
                    psum = proj_psum.tile([128, EW], F32)
                    for dch in range(8):
                        lhsT = xt[:, dch, tsub * 128:(tsub + 1) * 128]
                        if c >= 4:
                            nc.tensor.matmul(
                                psum[:, 0:512], lhsT, w_sb[:, dch, 0:512],
                                start=(dch == 0), stop=(dch == 7),
                            )
                        else:  # halo rows need only k,v
                            nc.tensor.matmul(
                                psum[:, 256:512], lhsT, w_sb[:, dch, 256:512],
                                start=(dch == 0), stop=(dch == 7),
                            )
                        nc.tensor.matmul(
                            psum[:, 512:EW], lhsT, w_sb[:, dch, 512:EW],
                            start=(dch == 0), stop=(dch == 7),
                        )
                    # psum segments: q0 q1 k0 k1 v0 v1, each [128, 128]
                    psum6 = psum.rearrange("p (s d) -> p s d", s=6)

                    # RMS-norm scales (halo chunks skip the q segments).
                    # Square on ACT with fused per-segment row-sum accumulation.
                    s0 = 0 if c >= 4 else 2
                    sq = stageA.tile([128, 6, HD], F32)
                    ssum = smallA.tile([128, 6], F32)
                    for sg in range(s0, 6):
                        nc.scalar.activation(
                            sq[:, sg, :], psum6[:, sg, :], AF.Square,
                            accum_out=ssum[:, sg:sg + 1],
                        )
                    # rms for q,k (eps bias) and v (lam0 folded via scale/bias)
                    rms = smallA.tile([128, 6], F32)
                    nc.scalar.activation(rms[:, s0:4], ssum[:, s0:4], AF.Sqrt,
                                         bias=eps_sb, scale=1.0 / HD)
                    nc.scalar.activation(rms[:, 4:6], ssum[:, 4:6], AF.Sqrt,
                                         bias=lam_sb[:, 3:4],
                                         scale=lam_sb[:, 2:3])
                    rs = smallA.tile([128, 6], F32)
                    nc.vector.reciprocal(rs[:, s0:6], rms[:, s0:6])

                    # normalize segments in one DVE op -> staging (f32r)
                    st6 = stageA.tile([128, 6, HD], F32R)
                    nc.vector.tensor_tensor(
                        out=st6[:, s0:6, :], in0=psum6[:, s0:6, :],
                        in1=rs[:, s0:6, None].to_broadcast([128, 6 - s0, HD]),
                        op=AOP.mult,
                    )
                    st6f = st6.bitcast(F32)

                    # v = lam1 * ve + v_normed (gpsimd; all-SBUF).
                    # Pool has no TensorScalarPtr, so use two tensor_tensor
                    # ops with a broadcast lam1 operand.
                    vel = stageA.tile([128, 2, HD], F32, name="vel")
                    nc.gpsimd.tensor_tensor(
                        out=vel, in0=vet[:, tsub, :].rearrange("p (h d) -> p h d", h=2),
                        in1=lam_sb[:, 1:2, None].to_broadcast([128, 2, HD]),
                        op=AOP.mult,
                    )
                    for h in range(HPC):
                        nc.vector.tensor_tensor(
                            out=vbf[h][:, c, :], in0=vel[:, h, :],
                            in1=st6f[:, 4 + h, :], op=AOP.add,
                        )

                    # rope on q,k (dims 0:32 rotate with dims 64:96); gpsimd
                    nseg = 4 - s0
                    cs = cos_sb[:, c:c + 1, :].to_broadcast([128, nseg, 32])
                    sn = sin_sb[:, c:c + 1, :].to_broadcast([128, nseg, 32])
                    x1 = st6f[:, s0:4, 0:32]
                    x2 = st6f[:, s0:4, 64:96]
                    t1 = stageA.tile([128, 4, 32], F32)
                    t2 = stageA.tile([128, 4, 32], F32)
                    t3 = stageA.tile([128, 4, 32], F32)
                    t4 = stageA.tile([128, 4, 32], F32)
                    nc.vector.tensor_tensor(out=t1[:, s0:4, :], in0=x1, in1=cs, op=AOP.mult)
                    nc.vector.tensor_tensor(out=t2[:, s0:4, :], in0=x2, in1=sn, op=AOP.mult)
                    nc.gpsimd.tensor_tensor(out=t3[:, s0:4, :], in0=x1, in1=sn, op=AOP.mult)
                    nc.gpsimd.tensor_tensor(out=t4[:, s0:4, :], in0=x2, in1=cs, op=AOP.mult)
                    nc.vector.tensor_add(st6[:, s0:4, 0:32], t1[:, s0:4, :], t2[:, s0:4, :])
                    nc.vector.tensor_sub(st6[:, s0:4, 64:96], t4[:, s0:4, :], t3[:, s0:4, :])

                    # transpose q,k into [dd, t] persistent buffers (f32r)
                    for h in range(HPC):
                        if c >= 4:  # q exists only for own rows
                            tq = tp_psum.tile([128, 128], F32R, name="tq", tag="tp")
                            nc.tensor.transpose(tq, st6[:, h, :], identity)
                            nc.vector.tensor_copy(
                                out=qT[h][:, (c - 4) * 128:(c - 3) * 128], in_=tq)
                        tk = tp_psum.tile([128, 128], F32R, name="tk", tag="tp")
                        nc.tensor.transpose(tk, st6[:, 2 + h, :], identity)
                        nc.vector.tensor_copy(out=kT[h][:, c * 128:(c + 1) * 128],
                                              in_=tk)

        nc.sync.dma_start(
            out=wo_sb, in_=woT.rearrange("(a p) e -> p a e", p=128).bitcast(F32R))
        nc.sync.dma_start(out=pad_r, in_=padcnt.rearrange("(a t) -> a t", a=1))

        # ====== Phase B+C: banded attention with interleaved out-projection ===
        with (
            tc.tile_pool(name="pm_pool", bufs=3) as pm_pool,
            tc.tile_pool(name="smallB", bufs=8) as smallB,
            tc.tile_pool(name="o_out", bufs=4) as o_out,
            tc.tile_pool(name="sc_psum", bufs=3, space="PSUM") as sc_psum,
            tc.tile_pool(name="sum_psum", bufs=1, space="PSUM") as sum_psum,
            tc.tile_pool(name="y_psum", bufs=2, space="PSUM") as y_psum,
            tc.tile_pool(name="o_psum", bufs=2, space="PSUM") as o_psum,
        ):
            def oproj_window(tw):
                # out[:, 512tw:512tw+512] = sum_h woT_h^T @ yT_h window
                for ec in range(8):
                    ops = o_psum.tile([128, 512], F32, name="ops")
                    for h in range(HPC):
                        nc.tensor.matmul(
                            ops,
                            wo_sb[:, h, ec * 128:(ec + 1) * 128],
                            yT[h][:, tw * 512:(tw + 1) * 512],
                            start=(h == 0), stop=(h == HPC - 1),
                            skip_group_check=True,
                        )
                    ot = o_out.tile([128, 512], F32, name="ot")
                    if ec % 2 == 0:
                        nc.scalar.copy(out=ot, in_=ops)
                    else:
                        nc.vector.tensor_copy(out=ot, in_=ops)
                    nc.sync.dma_start(
                        out=outT[ec * 128:(ec + 1) * 128,
                                 tw * 512:(tw + 1) * 512],
                        in_=ot,
                    )

            for pr in range(NPR):
                for h in range(HPC):
                    qs = qT[h][:, pr * 256:(pr + 1) * 256]
                    pm = pm_pool.tile([128, NPC, 256], F32R)
                    sums = sum_psum.tile([1, 256], F32, name="sums")
                    yps = y_psum.tile([128, 256], F32, name="yps")
                    # masked chunk pairs first so the final accumulation
                    # tail has no Pool mask op on its critical path
                    for i, wp in enumerate((0, 2, 1)):  # chunk pairs
                        sc = sc_psum.tile([128, 2, 256], F32, name="sc", tag="sc")
                        for j in range(2):
                            wc = 2 * wp + j
                            nc.tensor.matmul(
                                sc[:, j, :],
                                kT[h][:, (2 * pr + wc) * 128:(2 * pr + wc + 1) * 128],
                                qs, start=True, stop=True, skip_group_check=True,
                            )
                        nc.scalar.activation(pm[:, 2 * wp:2 * wp + 2, :], sc,
                                             AF.Exp, scale=ATTN_SCALE)
                        if wp == 0:
                            nc.vector.tensor_tensor(
                                out=pm[:, 0:2, :], in0=pm[:, 0:2, :].bitcast(F32),
                                in1=maskA, op=AOP.mult)
                        elif wp == 2:
                            nc.vector.tensor_tensor(
                                out=pm[:, 4:6, :], in0=pm[:, 4:6, :].bitcast(F32),
                                in1=maskB, op=AOP.mult)
                        for j in range(2):
                            wc = 2 * wp + j
                            nc.tensor.matmul(
                                sums, ones_col, pm[:, wc, :],
                                start=(i == 0 and j == 0),
                                stop=(i == 2 and j == 1),
                                skip_group_check=True,
                            )
                            nc.tensor.matmul(
                                yps, vbf[h][:, 2 * pr + wc, :], pm[:, wc, :],
                                start=(i == 0 and j == 0),
                                stop=(i == 2 and j == 1),
                                skip_group_check=True,
                            )
                    with tc.high_priority(offset=40):
                        sums2 = smallB.tile([1, 256], F32)
                        nc.vector.tensor_sub(sums2, sums,
                                             pad_r[:, pr * 256:(pr + 1) * 256])
                        recip = smallB.tile([1, 256], F32)
                        nc.vector.reciprocal(recip, sums2)
                        # broadcast 1/sum across partitions on the Pool engine
                        bc_sb = smallB.tile([128, 256], F32, name="bc_sb")
                        nc.gpsimd.partition_broadcast(bc_sb, recip)
                    # evacuate with the 1/sum normalization fused (cast f32r)
                    nc.vector.tensor_tensor(
                        out=yT[h][:, pr * 256:(pr + 1) * 256],
                        in0=yps, in1=bc_sb, op=AOP.mult)
                if pr % 2 == 1:
                    oproj_window(pr // 2)


_NC_CACHE = None


def _get_nc():
    global _NC_CACHE
    if _NC_CACHE is None:
        _NC_CACHE = build_kernel()
    return _NC_CACHE


def _rope_tables(positions):
    keep = HD // 4
    active = (1.0 / ROPE_BASE) ** np.linspace(0.0, 1.0, keep, dtype=np.float32)
    theta = positions[:, None].astype(np.float32) * active[None, :]  # [n, 32]
    return np.cos(theta).astype(np.float32), np.sin(theta).astype(np.float32)


def make_in_maps(x, ve, lambdas, qkvo_w):
    """Build the 8 per-core input maps from full inputs (host-side sharding)."""
    x2 = x.reshape(T, DIM)
    ve2 = ve.reshape(T, DIM)
    qw, kw, vw, ow = qkvo_w[0], qkvo_w[1], qkvo_w[2], qkvo_w[3]

    in_maps = []
    for c in range(8):
        s, g = divmod(c, G)
        h0, h1 = HPC * g, HPC * g + 1
        lo = TC * s - WINDOW  # first k/v row (may be negative -> zero pad)
        hi = TC * s + TC

        # xT slice with zero pad
        xs = np.zeros((TK, DIM), np.float32)
        src_lo = max(lo, 0)
        xs[src_lo - lo:, :] = x2[src_lo:hi, :]
        xTc = np.ascontiguousarray(xs.T)

        # fused qkv weight, transposed: cols = q0 q1 k0 k1 v0 v1
        wcols = []
        for wmat in (qw, kw, vw):
            for h in (h0, h1):
                wcols.append(wmat[h * HD:(h + 1) * HD, :].T)
        wqkvT = np.ascontiguousarray(np.concatenate(wcols, axis=1))

        woT = np.ascontiguousarray(ow[:, h0 * HD:(h1 + 1) * HD].T)

        ves = np.zeros((TK, HPC * HD), np.float32)
        ves[src_lo - lo:, :] = ve2[src_lo:hi, h0 * HD:(h1 + 1) * HD]

        pos = np.clip(np.arange(lo, hi), 0, None)
        cosT, sinT = _rope_tables(pos)

        l0, l1 = float(lambdas[0]), float(lambdas[1])
        lam_row = np.array([l0, l1, 1.0 / (HD * l0 * l0), EPS / (l0 * l0)],
                           np.float32)
        lam = np.tile(lam_row.reshape(1, 4), (128, 1)).astype(np.float32)

        pc = np.zeros(TC, np.float32)
        if s == 0:
            i = np.arange(TC)
            pc = np.maximum(0.0, WINDOW - 1.0 - i).astype(np.float32)

        in_maps.append({
            "xT": xTc, "wqkvT": wqkvT, "woT": woT, "ve": ves,
            "cosT": cosT, "sinT": sinT, "lam": lam, "padcnt": pc,
        })
    return in_maps


def kernel(x, ve, lambdas, qkvo_w, window):
    assert int(window) == WINDOW
    x = np.asarray(x, np.float32)
    ve = np.asarray(ve, np.float32)
    lambdas = np.asarray(lambdas, np.float32)
    qkvo_w = np.asarray(qkvo_w, np.float32)

    nc = _get_nc()
    in_maps = make_in_maps(x, ve, lambdas, qkvo_w)
    res = run_bass_kernel_spmd(nc, in_maps, core_ids=list(range(8)))

    outT_full = np.zeros((DIM, T), np.float32)
    for c in range(8):
        s = c // G
        outT_full[:, TC * s:TC * (s + 1)] += res.results[c]["outT"]
    return np.ascontiguousarray(outT_full.T).reshape(1, T, DIM)


if __name__ == "__main__":
    nc = _get_nc()
    print("kernel built ok")
